# revision 1
# baseline (speedup 1.0000x reference)
"""Trainium2 Bass kernel for nn_CPI_CLS_49478023250092 (gnn_message_passing).

Strategy (8 cores, SPMD):
  - GNN: adjacency row-sharded; each core holds A_c.T (4096x512) resident in
    SBUF, computes delta.T = (A_c @ hs).T = sum_k hs_chunk.T @ A_cT_chunk on
    the tensor engine; per-layer AllGather of the [10,512] delta recovers the
    full xs.T on every core.  3 layers.
  - Protein conv: L-sharded with 33-col halos (zero at global edges).  The
    23x23 conv over a [L,10] image needs no width padding (|w-j|<=9<11), so
    each output tile is TWO accumulating matmuls against a 12-shift stacked
    image X12 [120, L] (partition block p = image shifted by p columns).
  - Attention + fusion MLP on-device; tiny AllReduces for compound/protein.
  - Host side does only data movement: embedding gathers, sharding,
    transposition, Toeplitz construction, dtype casts.
"""

import sys
import os

for _p in ("/opt/trn_rl_repo",):
    if _p not in sys.path and os.path.isdir(_p):
        sys.path.insert(0, _p)

import numpy as np
import ml_dtypes

import concourse.bacc as bacc
import concourse.mybir as mybir
from concourse import tile
from concourse.bass_utils import run_bass_kernel_spmd

BF16 = ml_dtypes.bfloat16

NCORES = 8
NA = 4096          # atoms
D = 10             # embed dim
L = 65536          # words
KK = 23            # conv kernel
PAD = 11
R = NA // NCORES   # 512 adjacency rows per core
NCH = NA // 128    # 32 k-chunks
LC = L // NCORES   # 8192 conv columns per core
HALO = 33
LBUF = LC + 2 * HALO   # 8258
T = 512            # free-dim tile

F32 = mybir.dt.float32
BF = mybir.dt.bfloat16

# ---- smalls layout (f32 [128, 140]) ----
# cols 0-29   : wgT[l] [11,10] at cols 10l   (W_gnn_w[l].T stacked with bias row)
# cols 30-39  : watT f32 [10,10]
# col  40     : batt [10,1]
# cols 41-60  : woa0 [10,20] = W_out0[:, :10].T
# cols 61-80  : wob0 [10,20] = W_out0[:, 10:].T
# col  81     : bo0 [20,1]
# cols 82-101 : woT1 [20,20]
# cols 102-121: woT2 [20,20]
# col 122     : bo1 ; col 123 : bo2
# cols 124-125: wiT [20,2]
# col  126    : bi [2,1]
# cols 128-137: ones_sc [1,10] at partition 0 (value 1/65536)
SM_COLS = 140
# ---- gm layout (bf16 [120, 80]) ----
# cols 20l+0..9  : G0_l [120,10] ; cols 20l+10..19 : G1_l [110,10] (padded)
# cols 60-69     : watT bf16 [10,10] (partitions 0-9)
GM_COLS = 80

_BUILD_CACHE = {}


def _conv_spans():
    """Per conv layer (1..3): (in_lo, in_hi, out_lo, out_hi) in buffer coords."""
    spans = []
    for l in (1, 2, 3):
        in_lo = 11 * (l - 1)
        in_hi = LBUF - 11 * (l - 1)
        out_lo = 11 * l
        out_hi = LBUF - 11 * l
        spans.append((in_lo, in_hi, out_lo, out_hi))
    return spans


def _tiles(lo, hi, step):
    out = []
    c = lo
    while c < hi:
        out.append((c, min(step, hi - c)))
        c += step
    return out


def build_program():
    stage = int(os.environ.get("K_STAGE", "8"))
    key = ("nc", stage)
    if key in _BUILD_CACHE:
        return _BUILD_CACHE[key]

    nc = bacc.Bacc("TRN2", target_bir_lowering=False, debug=False,
                   num_devices=NCORES)

    xsT0 = nc.dram_tensor("xsT0", [11, NA], F32, kind="ExternalInput").ap()
    a_t = nc.dram_tensor("a_t", [NA, R], F32, kind="ExternalInput").ap()
    wsT = nc.dram_tensor("wsT", [D, LBUF], BF, kind="ExternalInput").ap()
    gm = nc.dram_tensor("gm", [120, GM_COLS], BF, kind="ExternalInput").ap()
    smalls = nc.dram_tensor("smalls", [128, SM_COLS], F32,
                            kind="ExternalInput").ap()
    out_d = nc.dram_tensor("out", [1, 2], F32, kind="ExternalOutput").ap()

    spans = _conv_spans()
    rg = [list(range(NCORES))]

    with tile.TileContext(nc) as tc:
        with (
            tc.tile_pool(name="const", bufs=1) as constp,
            tc.tile_pool(name="abuf", bufs=1) as abufp,
            tc.tile_pool(name="ximg", bufs=1) as ximgp,
            tc.tile_pool(name="x12", bufs=1) as x12p,
            tc.tile_pool(name="hs", bufs=1) as hsp_pool,
            tc.tile_pool(name="dl", bufs=2) as dlp,
            tc.tile_pool(name="att", bufs=3) as attp,
            tc.tile_pool(name="misc", bufs=2) as miscp,
            tc.tile_pool(name="ps_hs", bufs=2, space="PSUM") as ps_hs,
            tc.tile_pool(name="ps_dl", bufs=1, space="PSUM") as ps_dl,
            tc.tile_pool(name="ps_cv", bufs=3, space="PSUM") as ps_cv,
            tc.tile_pool(name="ps_sm", bufs=1, space="PSUM") as ps_sm,
            tc.tile_pool(name="ps_wr", bufs=1, space="PSUM") as ps_wr,
            tc.tile_pool(name="dram", bufs=1, space="DRAM") as dram,
        ):
            # ---------------- load phase ----------------
            sm = constp.tile([128, SM_COLS], F32, tag="sm")
            nc.sync.dma_start(sm[:], smalls[:])
            gmt = constp.tile([120, GM_COLS], BF, tag="gm")
            nc.sync.dma_start(gmt[:], gm[:])
            xsT = constp.tile([11, NA], F32, tag="xsT")
            nc.sync.dma_start(xsT[:], xsT0[:])
            ximg = ximgp.tile([D, LBUF], BF, tag="ximg")
            nc.sync.dma_start(ximg[:], wsT[:])

            a_sb = abufp.tile([128, NCH * T], F32, tag="a")
            for c in range(NCH):
                nc.sync.dma_start(a_sb[:, c * T:(c + 1) * T],
                                  a_t[c * 128:(c + 1) * 128, :])

            x12 = x12p.tile([120, LBUF], BF, tag="x12")

            # collective bounce buffers
            cc_in = [dram.tile([D, T], F32, tag=f"ccin{i}",
                               name=f"ccin{i}") for i in range(2)]
            cc_out = [dram.tile([8 * D, T], F32, tag=f"ccout{i}",
                                name=f"ccout{i}") for i in range(2)]
            ar_c_in = dram.tile([D, 8], F32, tag="arcin")
            ar_c_out = dram.tile([D, 8], F32, tag="arcout")
            ar_p_in = dram.tile([D, 8], F32, tag="arpin")
            ar_p_out = dram.tile([D, 8], F32, tag="arpout")

            wgT = [sm[0:11, 10 * l:10 * l + 10] for l in range(3)]
            watT = sm[0:D, 30:40]
            batt = sm[0:D, 40:41]
            watT_bf = gmt[0:D, 60:70]

            def build_x12(l):
                in_lo, in_hi, _, _ = spans[l - 1]
                src = ximg
                for p in range(12):
                    nc.sync.dma_start(
                        x12[10 * p:10 * p + 10, in_lo:in_hi - p],
                        src[:, in_lo + p:in_hi])

            def conv_layer(l, cbias):
                in_lo, in_hi, out_lo, out_hi = spans[l - 1]
                g0 = gmt[0:120, 20 * (l - 1):20 * (l - 1) + 10]
                g1 = gmt[0:110, 20 * (l - 1) + 10:20 * (l - 1) + 20]
                for (b0, tw) in _tiles(out_lo, out_hi, T):
                    ps = ps_cv.tile([D, T], F32, tag="cv")
                    nc.tensor.matmul(ps[:, :tw], g0,
                                     x12[0:120, b0 - 11:b0 - 11 + tw],
                                     start=True, stop=False)
                    nc.tensor.matmul(ps[:, :tw], g1,
                                     x12[0:110, b0 + 1:b0 + 1 + tw],
                                     start=False, stop=True)
                    nc.scalar.activation(ximg[:, b0:b0 + tw], ps[:, :tw],
                                         mybir.ActivationFunctionType.Relu,
                                         bias=cbias[l - 1])

            # conv biases: scalar per layer, baked as [D,1] columns
            _cb_cols = (138, 139, 127)
            cbias = [sm[0:D, cc:cc + 1] for cc in _cb_cols]

            def gnn_layer(l):
                """hs matmuls + delta accumulation; returns delta psum."""
                hs_sb = hsp_pool.tile([128, NCH * D], F32, tag="hs")
                for c in range(NCH):
                    hp = ps_hs.tile([128, D], F32, tag="hsps")
                    nc.tensor.matmul(hp[:], xsT[:, 128 * c:128 * (c + 1)],
                                     wgT[l])
                    nc.scalar.activation(hs_sb[:, D * c:D * (c + 1)], hp[:],
                                         mybir.ActivationFunctionType.Relu)
                dl_ps = ps_dl.tile([D, T], F32, tag="dl")
                for c in range(NCH):
                    nc.tensor.matmul(dl_ps[:], hs_sb[:, D * c:D * (c + 1)],
                                     a_sb[:, T * c:T * (c + 1)],
                                     start=(c == 0), stop=(c == NCH - 1))
                return dl_ps

            def apply_delta(idx):
                """DMA gathered deltas back and add into xsT."""
                dT = dlp.tile([D, NA], F32, tag="dT")
                nc.sync.dma_start(
                    dT[:].rearrange("j (r n) -> j r n", r=NCORES),
                    cc_out[idx][:].rearrange("(r j) n -> j r n", j=D))
                nc.vector.tensor_add(xsT[0:D, :], xsT[0:D, :], dT[:])

            def stage_delta(dl_ps, idx):
                dcp = dlp.tile([D, T], F32, tag="dcp")
                nc.scalar.activation(dcp[:], dl_ps[:],
                                     mybir.ActivationFunctionType.Copy)
                nc.sync.dma_start(cc_in[idx][:], dcp[:])

            probes = []

            # ---------------- GNN L1 ----------------
            dl1 = gnn_layer(0)
            stage_delta(dl1, 0)
            nc.gpsimd.collective_compute(
                "AllGather", mybir.AluOpType.bypass,
                ins=[cc_in[0].opt()], outs=[cc_out[0].opt()],
                replica_groups=rg)

            if stage >= 2:
                # conv L1 while AG1 is in flight
                build_x12(1)
                conv_layer(1, cbias)

            apply_delta(0)
            probes.append(("f32", xsT[0:1, 0:1]))

            if stage >= 3:
                dl2 = gnn_layer(1)
                stage_delta(dl2, 1)
                nc.gpsimd.collective_compute(
                    "AllGather", mybir.AluOpType.bypass,
                    ins=[cc_in[1].opt()], outs=[cc_out[1].opt()],
                    replica_groups=rg)

            if stage >= 4:
                build_x12(2)
                conv_layer(2, cbias)

            if stage >= 3:
                apply_delta(1)

            if stage >= 5:
                dl3 = gnn_layer(2)
                r1 = miscp.tile([D, 1], F32, tag="r1")
                nc.vector.tensor_reduce(r1[:], xsT[0:D, :],
                                        axis=mybir.AxisListType.X,
                                        op=mybir.AluOpType.add)
                r2 = miscp.tile([D, 1], F32, tag="r2")
                nc.vector.tensor_reduce(r2[:], dl3[:],
                                        axis=mybir.AxisListType.X,
                                        op=mybir.AluOpType.add)
                part_c = miscp.tile([D, 8], F32, tag="pc")
                nc.vector.memset(part_c[:], 0.0)
                nc.vector.tensor_scalar_mul(r2[:], r2[:], 1.0 / NA)
                nc.vector.scalar_tensor_tensor(
                    part_c[:, 0:1], r1[:], 1.0 / (NCORES * NA),
                    r2[:], op0=mybir.AluOpType.mult, op1=mybir.AluOpType.add)
                nc.sync.dma_start(ar_c_in[:], part_c[:])
                nc.gpsimd.collective_compute(
                    "AllReduce", mybir.AluOpType.add,
                    ins=[ar_c_in.opt()], outs=[ar_c_out.opt()],
                    replica_groups=rg)

            if stage >= 6:
                build_x12(3)
                conv_layer(3, cbias)
            if stage >= 2:
                probes.append(("bf", ximg[0:1, HALO + 1:HALO + 2]))

            if stage >= 5:
                comp = miscp.tile([D, 1], F32, tag="comp")
                nc.sync.dma_start(comp[:], ar_c_out[:, 0:1])
                probes.append(("f32", comp[0:1, 0:1]))

            sub = int(os.environ.get("K_SUB", "5"))
            if stage >= 7:
                h_ps = ps_sm.tile([20, 1], F32, tag="tiny")
                nc.tensor.matmul(h_ps[0:D, :], watT, comp[:])
                h_sb = miscp.tile([D, 1], F32, tag="hsb")
                nc.scalar.activation(h_sb[:], h_ps[0:D, :],
                                     mybir.ActivationFunctionType.Relu,
                                     bias=batt)
                ones_sc = sm[0:1, 128:138]
                pp = miscp.tile([D, 16], F32, tag="pp")
                pp2 = miscp.tile([1, 16], F32, tag="pp2")
                NT = LC // T  # 16
                for t in range(NT):
                    b0 = HALO + t * T
                    ps1 = ps_cv.tile([D, T], F32, tag="cv")
                    nc.tensor.matmul(ps1[:], watT_bf, ximg[:, b0:b0 + T])
                    hsp = attp.tile([D, T], F32, tag="hsp")
                    nc.scalar.activation(hsp[:], ps1[:],
                                         mybir.ActivationFunctionType.Relu,
                                         bias=batt)
                    if sub < 2:
                        nc.vector.tensor_reduce(pp[:, t:t + 1], hsp[:],
                                                axis=mybir.AxisListType.X,
                                                op=mybir.AluOpType.add)
                        continue
                    ps2 = ps_wr.tile([1, T], F32, tag="wr")
                    nc.tensor.matmul(ps2[:], h_sb[:], hsp[:])
                    wr = attp.tile([1, T], F32, tag="wrs")
                    nc.scalar.activation(wr[:], ps2[:],
                                         mybir.ActivationFunctionType.Tanh)
                    if sub < 3:
                        nc.vector.tensor_reduce(pp2[:, t:t + 1], wr[:],
                                                axis=mybir.AxisListType.X,
                                                op=mybir.AluOpType.add)
                        nc.vector.tensor_reduce(pp[:, t:t + 1], hsp[:],
                                                axis=mybir.AxisListType.X,
                                                op=mybir.AluOpType.add)
                        continue
                    ps3 = ps_cv.tile([D, T], F32, tag="cv")
                    nc.tensor.matmul(ps3[:], ones_sc, wr[:])
                    if sub < 4:
                        nc.vector.tensor_reduce(pp[:, t:t + 1], ps3[:],
                                                axis=mybir.AxisListType.X,
                                                op=mybir.AluOpType.add)
                        nc.vector.tensor_reduce(pp2[:, t:t + 1], hsp[0:1, :],
                                                axis=mybir.AxisListType.X,
                                                op=mybir.AluOpType.add)
                        continue
                    wrep = attp.tile([D, T], F32, tag="wrep")
                    nc.scalar.activation(wrep[:], ps3[:],
                                         mybir.ActivationFunctionType.Copy)
                    ys = attp.tile([D, T], F32, tag="ys")
                    nc.vector.tensor_mul(ys[:], hsp[:], wrep[:])
                    nc.vector.tensor_reduce(pp[:, t:t + 1], ys[:],
                                            axis=mybir.AxisListType.X,
                                            op=mybir.AluOpType.add)
                part_p = miscp.tile([D, 8], F32, tag="ppad")
                nc.vector.memset(part_p[:], 0.0)
                nc.vector.tensor_reduce(part_p[:, 0:1], pp[:],
                                        axis=mybir.AxisListType.X,
                                        op=mybir.AluOpType.add)
                if sub >= 5:
                    nc.sync.dma_start(ar_p_in[:], part_p[:])
                    nc.gpsimd.collective_compute(
                        "AllReduce", mybir.AluOpType.add,
                        ins=[ar_p_in.opt()], outs=[ar_p_out.opt()],
                        replica_groups=rg)
                    prot = miscp.tile([D, 1], F32, tag="prot")
                    nc.sync.dma_start(prot[:], ar_p_out[:, 0:1])
                else:
                    prot = miscp.tile([D, 1], F32, tag="prot")
                    nc.scalar.activation(prot[:], part_p[:, 0:1],
                                         mybir.ActivationFunctionType.Copy)
                probes.append(("f32", prot[0:1, 0:1]))

            if stage >= 8:
                # ---------------- fusion MLP ----------------
                woa0 = sm[0:D, 41:61]
                wob0 = sm[0:D, 61:81]
                bo0 = sm[0:20, 81:82]
                woT1 = sm[0:20, 82:102]
                woT2 = sm[0:20, 102:122]
                bo1 = sm[0:20, 122:123]
                bo2 = sm[0:20, 123:124]
                wiT = sm[0:20, 124:126]
                bi = sm[0:2, 126:127]

                f_ps = ps_sm.tile([20, 1], F32, tag="tiny")
                nc.tensor.matmul(f_ps[:], woa0, comp[:], start=True,
                                 stop=False)
                nc.tensor.matmul(f_ps[:], wob0, prot[:], start=False,
                                 stop=True)
                cat1 = miscp.tile([20, 1], F32, tag="cat1")
                nc.scalar.activation(cat1[:], f_ps[:],
                                     mybir.ActivationFunctionType.Relu,
                                     bias=bo0)
                f_ps2 = ps_sm.tile([20, 1], F32, tag="tiny")
                nc.tensor.matmul(f_ps2[:], woT1, cat1[:])
                cat2 = miscp.tile([20, 1], F32, tag="cat2")
                nc.scalar.activation(cat2[:], f_ps2[:],
                                     mybir.ActivationFunctionType.Relu,
                                     bias=bo1)
                f_ps3 = ps_sm.tile([20, 1], F32, tag="tiny")
                nc.tensor.matmul(f_ps3[:], woT2, cat2[:])
                cat3 = miscp.tile([20, 1], F32, tag="cat3")
                nc.scalar.activation(cat3[:], f_ps3[:],
                                     mybir.ActivationFunctionType.Relu,
                                     bias=bo2)
                o_ps = ps_sm.tile([20, 1], F32, tag="tiny")
                nc.tensor.matmul(o_ps[0:2, :], wiT, cat3[:])
                o_sb = miscp.tile([2, 1], F32, tag="osb")
                nc.scalar.activation(o_sb[:], o_ps[0:2, :],
                                     mybir.ActivationFunctionType.Identity,
                                     bias=bi)
                nc.sync.dma_start(out_d[:], o_sb[:])
            else:
                # debug output: accumulate probes so nothing is dead code
                dbg = miscp.tile([1, 2], F32, tag="dbg")
                nc.vector.memset(dbg[:], 0.0)
                for kind, ap in probes:
                    t8 = miscp.tile([1, 1], F32, tag="t8")
                    nc.scalar.activation(t8[:], ap,
                                         mybir.ActivationFunctionType.Copy)
                    nc.vector.tensor_add(dbg[:, 0:1], dbg[:, 0:1], t8[:])
                nc.sync.dma_start(out_d[:], dbg[:])

    nc.compile()
    _BUILD_CACHE[key] = nc
    return nc


def _host_prep(fingerprints, adjacency, words, embed_fp, embed_word,
               W_gnn_w, W_gnn_b, W_cnn_w, W_cnn_b, W_att_w, W_att_b,
               W_out_w, W_out_b, W_int_w, W_int_b):
    f32 = np.float32
    fingerprints = np.asarray(fingerprints).astype(np.int64)
    words = np.asarray(words).astype(np.int64)
    adjacency = np.asarray(adjacency, dtype=f32)
    embed_fp = np.asarray(embed_fp, dtype=f32)
    embed_word = np.asarray(embed_word, dtype=f32)
    W_gnn_w = np.asarray(W_gnn_w, dtype=f32)
    W_gnn_b = np.asarray(W_gnn_b, dtype=f32)
    W_cnn_w = np.asarray(W_cnn_w, dtype=f32)
    W_cnn_b = np.asarray(W_cnn_b, dtype=f32)
    W_att_w = np.asarray(W_att_w, dtype=f32)
    W_att_b = np.asarray(W_att_b, dtype=f32)
    W_out_w = np.asarray(W_out_w, dtype=f32)
    W_out_b = np.asarray(W_out_b, dtype=f32)
    W_int_w = np.asarray(W_int_w, dtype=f32)
    W_int_b = np.asarray(W_int_b, dtype=f32)

    # xsT0 [11, NA]: gathered compound embeddings, transposed + ones row
    xs0 = embed_fp[fingerprints]                       # [NA, D]
    xsT0 = np.zeros((11, NA), dtype=f32)
    xsT0[0:D] = xs0.T
    xsT0[D] = 1.0

    # adjacency row-shards, transposed: [NA, R] per core
    a_t = [np.ascontiguousarray(adjacency[c * R:(c + 1) * R, :].T)
           for c in range(NCORES)]

    # protein image shards with halo, transposed, bf16
    ws = embed_word[words]                             # [L, D]
    wspad = np.zeros((L + 2 * HALO, D), dtype=f32)
    wspad[HALO:HALO + L] = ws
    wsT = [np.ascontiguousarray(wspad[c * LC:c * LC + LBUF].T).astype(BF16)
           for c in range(NCORES)]

    # conv Toeplitz groups
    gmv = np.zeros((120, GM_COLS), dtype=f32)
    for l in range(3):
        ker = W_cnn_w[l, 0, 0]                         # [23, 23]
        g0 = np.zeros((120, D), dtype=f32)
        g1 = np.zeros((110, D), dtype=f32)
        for w in range(D):
            for j in range(D):
                kx = w - j + PAD
                for p in range(12):
                    g0[10 * p + w, j] = ker[p, kx]
                for p in range(11):
                    g1[10 * p + w, j] = ker[p + 12, kx]
        gmv[:, 20 * l:20 * l + 10] = g0
        gmv[0:110, 20 * l + 10:20 * l + 20] = g1
    gmv[0:D, 60:70] = W_att_w.T
    gmv = gmv.astype(BF16)

    sm = np.zeros((128, SM_COLS), dtype=f32)
    for l in range(3):
        sm[0:D, 10 * l:10 * l + 10] = W_gnn_w[l].T
        sm[D, 10 * l:10 * l + 10] = W_gnn_b[l]
    sm[0:D, 30:40] = W_att_w.T
    sm[0:D, 40] = W_att_b
    sm[0:D, 41:61] = W_out_w[0][:, 0:D].T
    sm[0:D, 61:81] = W_out_w[0][:, D:2 * D].T
    sm[0:20, 81] = W_out_b[0]
    sm[0:20, 82:102] = W_out_w[1].T
    sm[0:20, 102:122] = W_out_w[2].T
    sm[0:20, 122] = W_out_b[1]
    sm[0:20, 123] = W_out_b[2]
    sm[0:20, 124:126] = W_int_w.T
    sm[0:2, 126] = W_int_b
    sm[0:1, 128:138] = 1.0 / L
    sm[0:D, 138] = W_cnn_b[0]
    sm[0:D, 139] = W_cnn_b[1]
    sm[0:D, 127] = W_cnn_b[2]

    in_maps = []
    for c in range(NCORES):
        in_maps.append({
            "xsT0": xsT0,
            "a_t": a_t[c],
            "wsT": wsT[c],
            "gm": gmv,
            "smalls": sm,
        })
    return in_maps


def kernel(**inputs):
    in_maps = _host_prep(**inputs)
    nc = build_program()
    res = run_bass_kernel_spmd(nc, in_maps, list(range(NCORES)))
    return np.asarray(res.results[0]["out"], dtype=np.float32)



# revision 6
# speedup vs baseline: 1.4998x; 1.4998x over previous
"""Trainium2 Bass kernel for nn_CPI_CLS_49478023250092 (gnn_message_passing).

Strategy (8 cores, SPMD):
  - GNN: adjacency row-sharded; each core holds A_c.T (4096x512, bf16,
    pre-chunked on host into [128, 32*512]) resident in SBUF, computes
    delta.T = (A_c @ hs).T = sum_k hs_chunk.T @ A_cT_chunk on the tensor
    engine in bf16; per-layer AllGather of the bf16 [10,512] delta
    recovers the full xs.T on every core.  3 layers.
  - Protein conv: L-sharded with 33-col halos (zero at global edges).
    23x23 conv over a [L,10] image = TWO accumulating bf16 matmuls per
    512-col tile against a 12-shift stacked image X12 [120, L].
  - Attention: two-pass tail.  Pass 1 (overlaps the compound AllReduce):
    hsp = relu(W_att @ conv_out + b) for all 16 tiles.  Pass 2: 16
    weight matmuls into one [16,512] PSUM bank, one batched tanh, then
    per tile a broadcast matmul + fused multiply-reduce
    (tensor_tensor_reduce) accumulating the weighted mean.
  - Fusion MLP in f32 on every core; tiny AllReduces for compound and
    protein partial means.
  - Host side does only data movement: embedding gathers, sharding,
    transposition, Toeplitz construction, dtype casts.
"""

import sys
import os

for _p in ("/opt/trn_rl_repo",):
    if _p not in sys.path and os.path.isdir(_p):
        sys.path.insert(0, _p)

import numpy as np
import ml_dtypes

import concourse.bacc as bacc
import concourse.mybir as mybir
from concourse import tile
from concourse.bass_utils import run_bass_kernel_spmd

BF16 = ml_dtypes.bfloat16

NCORES = 8
NA = 4096          # atoms
D = 10             # embed dim
L = 65536          # words
KK = 23            # conv kernel
PAD = 11
R = NA // NCORES   # 512 adjacency rows per core
NCH = NA // 128    # 32 k-chunks
LC = L // NCORES   # 8192 conv columns per core
HALO = 33
LBUF = LC + 2 * HALO   # 8258
T = 512            # free-dim tile
NT = LC // T       # 16 attention tiles

F32 = mybir.dt.float32
BF = mybir.dt.bfloat16

# ---- smalls layout (f32 [128, 100]) ----
# cols 0-9   : watT f32 [10,10]
# col  10    : batt [10,1]
# cols 11-30 : woa0 [10,20] = W_out0[:, :10].T
# cols 31-50 : wob0 [10,20] = W_out0[:, 10:].T
# col  51    : bo0 [20,1]
# cols 52-71 : woT1 [20,20]
# cols 72-91 : woT2 [20,20]
# col 92     : bo1 ; col 93 : bo2
# cols 94-95 : wiT [20,2]
# col  96    : bi [2,1]
# cols 97-99 : conv biases [10,1] for layers 1..3
SM_COLS = 100
# ---- gm layout (bf16 [120, 110]) ----
# cols 20l+0..9  : G0_l [120,10] ; cols 20l+10..19 : G1_l [110,10] (padded)
# cols 60-69     : watT bf16 [10,10]
# cols 70+10l    : wgT_bf[l] [11,10] (W_gnn_w[l].T stacked with bias row)
# cols 100-109   : ones_sc [1,10] at partition 0 (value 1/65536)
GM_COLS = 110

_BUILD_CACHE = {}


def _conv_spans():
    """Per conv layer (1..3): (in_lo, in_hi, out_lo, out_hi) in buffer coords."""
    spans = []
    for l in (1, 2, 3):
        in_lo = 11 * (l - 1)
        in_hi = LBUF - 11 * (l - 1)
        out_lo = 11 * l
        out_hi = LBUF - 11 * l
        spans.append((in_lo, in_hi, out_lo, out_hi))
    return spans


def _tiles(lo, hi, step):
    out = []
    c = lo
    while c < hi:
        out.append((c, min(step, hi - c)))
        c += step
    return out


def build_program():
    # tensor_tensor_reduce hangs real HW (works in CoreSim) — keep it off
    TTR = os.environ.get("K_TTR", "0") == "1"
    VRELU = os.environ.get("K_VRELU", "1") == "1"  # DVE relu variants
    key = ("nc", TTR, VRELU)
    if key in _BUILD_CACHE:
        return _BUILD_CACHE[key]

    nc = bacc.Bacc("TRN2", target_bir_lowering=False, debug=False,
                   num_devices=NCORES)

    xsT0 = nc.dram_tensor("xsT0", [11, NA], BF, kind="ExternalInput").ap()
    a_p = nc.dram_tensor("a_p", [128, NCH * T], BF, kind="ExternalInput").ap()
    wsT = nc.dram_tensor("wsT", [D, LBUF], BF, kind="ExternalInput").ap()
    gm = nc.dram_tensor("gm", [120, GM_COLS], BF, kind="ExternalInput").ap()
    smalls = nc.dram_tensor("smalls", [128, SM_COLS], F32,
                            kind="ExternalInput").ap()
    out_d = nc.dram_tensor("out", [1, 2], F32, kind="ExternalOutput").ap()

    spans = _conv_spans()
    rg = [list(range(NCORES))]
    AF = mybir.ActivationFunctionType
    ALU = mybir.AluOpType

    with tile.TileContext(nc) as tc:
        with (
            tc.tile_pool(name="const", bufs=1) as constp,
            tc.tile_pool(name="abuf", bufs=1) as abufp,
            tc.tile_pool(name="ximg", bufs=1) as ximgp,
            tc.tile_pool(name="x12", bufs=1) as x12p,
            tc.tile_pool(name="hs", bufs=1) as hsp_pool,
            tc.tile_pool(name="dl", bufs=2) as dlp,
            tc.tile_pool(name="att", bufs=2) as attp,
            tc.tile_pool(name="misc", bufs=2) as miscp,
            tc.tile_pool(name="ps_hs", bufs=2, space="PSUM") as ps_hs,
            tc.tile_pool(name="ps_dl", bufs=1, space="PSUM") as ps_dl,
            tc.tile_pool(name="ps_cv", bufs=3, space="PSUM") as ps_cv,
            tc.tile_pool(name="ps_sm", bufs=1, space="PSUM") as ps_sm,
            tc.tile_pool(name="ps_wr", bufs=1, space="PSUM") as ps_wr,
            tc.tile_pool(name="dram", bufs=1, space="DRAM") as dram,
        ):
            # ---------------- load phase ----------------
            sm = constp.tile([128, SM_COLS], F32, tag="sm")
            nc.sync.dma_start(sm[:], smalls[:])
            gmt = constp.tile([120, GM_COLS], BF, tag="gm")
            nc.sync.dma_start(gmt[:], gm[:])
            xsT = constp.tile([11, NA], BF, tag="xsT")
            nc.sync.dma_start(xsT[:], xsT0[:])
            ximg = ximgp.tile([D, LBUF], BF, tag="ximg")
            nc.sync.dma_start(ximg[:], wsT[:])

            a_sb = abufp.tile([128, NCH * T], BF, tag="a")
            ADMA = 4  # chunks per DMA
            for c in range(0, NCH, ADMA):
                nc.sync.dma_start(a_sb[:, c * T:(c + ADMA) * T],
                                  a_p[:, c * T:(c + ADMA) * T])

            x12 = x12p.tile([120, LBUF], BF, tag="x12")

            # collective bounce buffers
            cc_in = [dram.tile([D, T], BF, tag=f"ccin{i}",
                               name=f"ccin{i}") for i in range(2)]
            cc_out = [dram.tile([8 * D, T], BF, tag=f"ccout{i}",
                                name=f"ccout{i}") for i in range(2)]
            ar_c_in = dram.tile([D, 8], F32, tag="arcin")
            ar_c_out = dram.tile([D, 8], F32, tag="arcout")
            ar_p_in = dram.tile([D, 8], F32, tag="arpin")
            ar_p_out = dram.tile([D, 8], F32, tag="arpout")

            wgT = [gmt[0:11, 70 + 10 * l:80 + 10 * l] for l in range(3)]
            watT = sm[0:D, 0:10]
            batt = sm[0:D, 10:11]
            watT_bf = gmt[0:D, 60:70]
            ones_bf = gmt[0:1, 100:110]
            cbias = [sm[0:D, 97 + i:98 + i] for i in range(3)]

            def build_x12(l):
                in_lo, in_hi, _, _ = spans[l - 1]
                for p in range(12):
                    nc.sync.dma_start(
                        x12[10 * p:10 * p + 10, in_lo:in_hi - p],
                        ximg[:, in_lo + p:in_hi])

            def conv_layer(l):
                in_lo, in_hi, out_lo, out_hi = spans[l - 1]
                g0 = gmt[0:120, 20 * (l - 1):20 * (l - 1) + 10]
                g1 = gmt[0:110, 20 * (l - 1) + 10:20 * (l - 1) + 20]
                for (b0, tw) in _tiles(out_lo, out_hi, T):
                    ps = ps_cv.tile([D, T], F32, tag="cv")
                    nc.tensor.matmul(ps[:, :tw], g0,
                                     x12[0:120, b0 - 11:b0 - 11 + tw],
                                     start=True, stop=False)
                    nc.tensor.matmul(ps[:, :tw], g1,
                                     x12[0:110, b0 + 1:b0 + 1 + tw],
                                     start=False, stop=True)
                    nc.scalar.activation(ximg[:, b0:b0 + tw], ps[:, :tw],
                                         AF.Relu, bias=cbias[l - 1])

            def gnn_layer(l):
                """hs matmuls + delta accumulation; returns delta psum."""
                hs_sb = hsp_pool.tile([128, NCH * D], BF, tag="hs")
                for c in range(NCH):
                    hp = ps_hs.tile([128, D], F32, tag="hsps")
                    nc.tensor.matmul(hp[:], xsT[:, 128 * c:128 * (c + 1)],
                                     wgT[l])
                    dst = hs_sb[:, D * c:D * (c + 1)]
                    if VRELU and c % 2 == 1:
                        nc.vector.tensor_scalar_max(dst, hp[:], 0.0)
                    else:
                        nc.scalar.activation(dst, hp[:], AF.Relu)
                dl_ps = ps_dl.tile([D, T], F32, tag="dl")
                for c in range(NCH):
                    nc.tensor.matmul(dl_ps[:], hs_sb[:, D * c:D * (c + 1)],
                                     a_sb[:, T * c:T * (c + 1)],
                                     start=(c == 0), stop=(c == NCH - 1))
                return dl_ps

            def stage_delta(dl_ps, idx):
                dcp = dlp.tile([D, T], BF, tag="dcp")
                nc.scalar.activation(dcp[:], dl_ps[:], AF.Copy)
                nc.sync.dma_start(cc_in[idx][:], dcp[:])

            def apply_delta(idx, accum_r1=None):
                """DMA gathered deltas back and add into xsT."""
                dT = dlp.tile([D, NA], BF, tag="dT")
                nc.sync.dma_start(
                    dT[:].rearrange("j (r n) -> j r n", r=NCORES),
                    cc_out[idx][:].rearrange("(r j) n -> j r n", j=D))
                if accum_r1 is not None and TTR:
                    nc.vector.tensor_tensor_reduce(
                        xsT[0:D, :], xsT[0:D, :], dT[:], 1.0, 0.0,
                        op0=ALU.add, op1=ALU.add, accum_out=accum_r1)
                else:
                    nc.vector.tensor_add(xsT[0:D, :], xsT[0:D, :], dT[:])
                    if accum_r1 is not None:
                        nc.vector.tensor_reduce(accum_r1, xsT[0:D, :],
                                                axis=mybir.AxisListType.X,
                                                op=ALU.add)

            # ---------------- GNN L1 ----------------
            dl1 = gnn_layer(0)
            stage_delta(dl1, 0)
            nc.gpsimd.collective_compute(
                "AllGather", ALU.bypass,
                ins=[cc_in[0].opt()], outs=[cc_out[0].opt()],
                replica_groups=rg)

            # conv L1 while AG1 is in flight
            build_x12(1)
            conv_layer(1)

            apply_delta(0)

            # ---------------- GNN L2 ----------------
            dl2 = gnn_layer(1)
            stage_delta(dl2, 1)
            nc.gpsimd.collective_compute(
                "AllGather", ALU.bypass,
                ins=[cc_in[1].opt()], outs=[cc_out[1].opt()],
                replica_groups=rg)

            build_x12(2)
            conv_layer(2)

            r1 = miscp.tile([D, 1], F32, tag="r1")
            apply_delta(1, accum_r1=r1[:])

            # ---------------- GNN L3 + compound ----------------
            dl3 = gnn_layer(2)
            r2 = miscp.tile([D, 1], F32, tag="r2")
            nc.vector.tensor_reduce(r2[:], dl3[:],
                                    axis=mybir.AxisListType.X,
                                    op=ALU.add)
            part_c = miscp.tile([D, 8], F32, tag="pc")
            nc.vector.memset(part_c[:], 0.0)
            nc.vector.tensor_scalar_mul(r2[:], r2[:], 1.0 / NA)
            nc.vector.scalar_tensor_tensor(
                part_c[:, 0:1], r1[:], 1.0 / (NCORES * NA),
                r2[:], op0=ALU.mult, op1=ALU.add)
            nc.sync.dma_start(ar_c_in[:], part_c[:])
            nc.gpsimd.collective_compute(
                "AllReduce", ALU.add,
                ins=[ar_c_in.opt()], outs=[ar_c_out.opt()],
                replica_groups=rg)

            build_x12(3)
            conv_layer(3)

            # -------- attention pass 1: hsp for all tiles (no comp dep) ----
            hspA = attp.tile([D, NT * T], BF, tag="hspA")
            for t in range(NT):
                b0 = HALO + t * T
                ps1 = ps_cv.tile([D, T], F32, tag="cv")
                nc.tensor.matmul(ps1[:], watT_bf, ximg[:, b0:b0 + T])
                dst = hspA[:, t * T:(t + 1) * T]
                if VRELU and t % 2 == 1:
                    nc.vector.tensor_scalar(dst, ps1[:], batt, 0.0,
                                            op0=ALU.add, op1=ALU.max)
                else:
                    nc.scalar.activation(dst, ps1[:], AF.Relu, bias=batt)

            # -------- compound + h --------
            comp = miscp.tile([D, 1], F32, tag="comp")
            nc.sync.dma_start(comp[:], ar_c_out[:, 0:1])
            h_ps = ps_sm.tile([20, 1], F32, tag="tiny")
            nc.tensor.matmul(h_ps[0:D, :], watT, comp[:])
            h_sb = miscp.tile([D, 1], BF, tag="hsb")
            nc.scalar.activation(h_sb[:], h_ps[0:D, :], AF.Relu, bias=batt)

            # -------- attention pass 2 --------
            wrall = attp.tile([1, NT * T], BF, tag="wrall")
            pp = miscp.tile([D, NT], F32, tag="pp")
            ys = attp.tile([D, T], BF, tag="ys")
            for t in range(NT):
                wrps = ps_wr.tile([1, T], F32, tag="wr")
                nc.tensor.matmul(wrps[:], h_sb[:],
                                 hspA[:, t * T:(t + 1) * T])
                wr = wrall[0:1, t * T:(t + 1) * T]
                nc.scalar.activation(wr, wrps[:], AF.Tanh)
                ps3 = ps_cv.tile([D, T], F32, tag="cv")
                nc.tensor.matmul(ps3[:], ones_bf, wr)
                if TTR:
                    nc.vector.tensor_tensor_reduce(
                        ys[:], hspA[:, t * T:(t + 1) * T], ps3[:], 1.0, 0.0,
                        op0=ALU.mult, op1=ALU.add, accum_out=pp[:, t:t + 1])
                else:
                    nc.vector.tensor_mul(ys[:], hspA[:, t * T:(t + 1) * T],
                                         ps3[:])
                    nc.vector.tensor_reduce(pp[:, t:t + 1], ys[:],
                                            axis=mybir.AxisListType.X,
                                            op=ALU.add)

            part_p = miscp.tile([D, 8], F32, tag="ppad")
            nc.vector.memset(part_p[:], 0.0)
            nc.vector.tensor_reduce(part_p[:, 0:1], pp[:],
                                    axis=mybir.AxisListType.X,
                                    op=ALU.add)
            nc.sync.dma_start(ar_p_in[:], part_p[:])
            nc.gpsimd.collective_compute(
                "AllReduce", ALU.add,
                ins=[ar_p_in.opt()], outs=[ar_p_out.opt()],
                replica_groups=rg)
            prot = miscp.tile([D, 1], F32, tag="prot")
            nc.sync.dma_start(prot[:], ar_p_out[:, 0:1])

            # ---------------- fusion MLP ----------------
            woa0 = sm[0:D, 11:31]
            wob0 = sm[0:D, 31:51]
            bo0 = sm[0:20, 51:52]
            woT1 = sm[0:20, 52:72]
            woT2 = sm[0:20, 72:92]
            bo1 = sm[0:20, 92:93]
            bo2 = sm[0:20, 93:94]
            wiT = sm[0:20, 94:96]
            bi = sm[0:2, 96:97]

            f_ps = ps_sm.tile([20, 1], F32, tag="tiny")
            nc.tensor.matmul(f_ps[:], woa0, comp[:], start=True, stop=False)
            nc.tensor.matmul(f_ps[:], wob0, prot[:], start=False, stop=True)
            cat1 = miscp.tile([20, 1], F32, tag="cat1")
            nc.scalar.activation(cat1[:], f_ps[:], AF.Relu, bias=bo0)
            f_ps2 = ps_sm.tile([20, 1], F32, tag="tiny")
            nc.tensor.matmul(f_ps2[:], woT1, cat1[:])
            cat2 = miscp.tile([20, 1], F32, tag="cat2")
            nc.scalar.activation(cat2[:], f_ps2[:], AF.Relu, bias=bo1)
            f_ps3 = ps_sm.tile([20, 1], F32, tag="tiny")
            nc.tensor.matmul(f_ps3[:], woT2, cat2[:])
            cat3 = miscp.tile([20, 1], F32, tag="cat3")
            nc.scalar.activation(cat3[:], f_ps3[:], AF.Relu, bias=bo2)
            o_ps = ps_sm.tile([20, 1], F32, tag="tiny")
            nc.tensor.matmul(o_ps[0:2, :], wiT, cat3[:])
            o_sb = miscp.tile([2, 1], F32, tag="osb")
            nc.scalar.activation(o_sb[:], o_ps[0:2, :], AF.Identity, bias=bi)
            nc.sync.dma_start(out_d[:], o_sb[:])

    nc.compile()
    _BUILD_CACHE[key] = nc
    return nc


def _host_prep(fingerprints, adjacency, words, embed_fp, embed_word,
               W_gnn_w, W_gnn_b, W_cnn_w, W_cnn_b, W_att_w, W_att_b,
               W_out_w, W_out_b, W_int_w, W_int_b):
    f32 = np.float32
    fingerprints = np.asarray(fingerprints).astype(np.int64)
    words = np.asarray(words).astype(np.int64)
    adjacency = np.asarray(adjacency, dtype=f32)
    embed_fp = np.asarray(embed_fp, dtype=f32)
    embed_word = np.asarray(embed_word, dtype=f32)
    W_gnn_w = np.asarray(W_gnn_w, dtype=f32)
    W_gnn_b = np.asarray(W_gnn_b, dtype=f32)
    W_cnn_w = np.asarray(W_cnn_w, dtype=f32)
    W_cnn_b = np.asarray(W_cnn_b, dtype=f32)
    W_att_w = np.asarray(W_att_w, dtype=f32)
    W_att_b = np.asarray(W_att_b, dtype=f32)
    W_out_w = np.asarray(W_out_w, dtype=f32)
    W_out_b = np.asarray(W_out_b, dtype=f32)
    W_int_w = np.asarray(W_int_w, dtype=f32)
    W_int_b = np.asarray(W_int_b, dtype=f32)

    # xsT0 [11, NA]: gathered compound embeddings, transposed + ones row
    xs0 = embed_fp[fingerprints]                       # [NA, D]
    xsT0 = np.zeros((11, NA), dtype=f32)
    xsT0[0:D] = xs0.T
    xsT0[D] = 1.0
    xsT0 = xsT0.astype(BF16)

    # adjacency row-shards, transposed, pre-chunked for SBUF, bf16:
    # a_p[p, c*T + j] = A[core*R + j, c*128 + p]
    a_p = []
    for c in range(NCORES):
        at = np.ascontiguousarray(adjacency[c * R:(c + 1) * R, :].T)  # [NA,R]
        ap = at.reshape(NCH, 128, R).transpose(1, 0, 2).reshape(128, NCH * R)
        a_p.append(ap.astype(BF16))

    # protein image shards with halo, transposed, bf16
    ws = embed_word[words]                             # [L, D]
    wspad = np.zeros((L + 2 * HALO, D), dtype=f32)
    wspad[HALO:HALO + L] = ws
    wsT = [np.ascontiguousarray(wspad[c * LC:c * LC + LBUF].T).astype(BF16)
           for c in range(NCORES)]

    # conv Toeplitz groups
    gmv = np.zeros((120, GM_COLS), dtype=f32)
    for l in range(3):
        ker = W_cnn_w[l, 0, 0]                         # [23, 23]
        g0 = np.zeros((120, D), dtype=f32)
        g1 = np.zeros((110, D), dtype=f32)
        for w in range(D):
            for j in range(D):
                kx = w - j + PAD
                for p in range(12):
                    g0[10 * p + w, j] = ker[p, kx]
                for p in range(11):
                    g1[10 * p + w, j] = ker[p + 12, kx]
        gmv[:, 20 * l:20 * l + 10] = g0
        gmv[0:110, 20 * l + 10:20 * l + 20] = g1
    gmv[0:D, 60:70] = W_att_w.T
    for l in range(3):
        gmv[0:D, 70 + 10 * l:80 + 10 * l] = W_gnn_w[l].T
        gmv[D, 70 + 10 * l:80 + 10 * l] = W_gnn_b[l]
    gmv[0:1, 100:110] = 1.0 / L
    gmv = gmv.astype(BF16)

    sm = np.zeros((128, SM_COLS), dtype=f32)
    sm[0:D, 0:10] = W_att_w.T
    sm[0:D, 10] = W_att_b
    sm[0:D, 11:31] = W_out_w[0][:, 0:D].T
    sm[0:D, 31:51] = W_out_w[0][:, D:2 * D].T
    sm[0:20, 51] = W_out_b[0]
    sm[0:20, 52:72] = W_out_w[1].T
    sm[0:20, 72:92] = W_out_w[2].T
    sm[0:20, 92] = W_out_b[1]
    sm[0:20, 93] = W_out_b[2]
    sm[0:20, 94:96] = W_int_w.T
    sm[0:2, 96] = W_int_b
    for i in range(3):
        sm[0:D, 97 + i] = W_cnn_b[i]

    in_maps = []
    for c in range(NCORES):
        in_maps.append({
            "xsT0": xsT0,
            "a_p": a_p[c],
            "wsT": wsT[c],
            "gm": gmv,
            "smalls": sm,
        })
    return in_maps


def kernel(**inputs):
    in_maps = _host_prep(**inputs)
    nc = build_program()
    res = run_bass_kernel_spmd(nc, in_maps, list(range(NCORES)))
    return np.asarray(res.results[0]["out"], dtype=np.float32)
